# revision 1
# baseline (speedup 1.0000x reference)
"""EntityEncoder Trainium2 kernel.

Computes, for each (batch, sentence j): ragged per-entity span mean-pool over
token embeddings, then a Linear projection:

    pooled[b, j, k, :] = mean(zipped_entity[b, j, start_kj:end_kj, :])
    out[b, j*K+k, :]   = pooled @ W + b

Strategy (8 NeuronCores, data-parallel over batch, memory-bound):
  - Each core gets 4 samples of zipped_entity; operands are cast to f16
    (halves HBM traffic; ~3.5e-4 rel err) and streamed at 1 col/cycle on
    the PE array. z loads alternate the two HWDGE rings (sync/scalar
    sequencers) so per-DMA dispatch overhead overlaps transfers.
  - Span masks (built host-side from the tiny sep-index tensor) turn the
    ragged mean-pool into a mask^T @ Z matmul on the tensor engine:
        sums[e, d] = sum_l mask[l, e] * Z[l, d]
    accumulated over the sample's 128-row L-chunks into PSUM. Trailing
    chunks beyond each sentence's last separator are skipped entirely
    (compile-time specialization from the actual sep values; samples are
    assigned to cores to balance the per-slot chunk maxima).
  - sums [16, 768] goes PSUM->SBUF with the two D-halves copied in
    parallel (DVE + ScalarE), is PE-transposed to [768, 16] chunks, and
    scaled by 1/count in [128, 3, 16] broadcast multiplies (1/0 = inf
    reproduces the reference's 0/0 -> NaN exactly). The final Linear
    contracts D on partitions: out = pooledT.T @ W + b. The last block's
    z arrives as per-chunk DMAs to shorten the end-of-kernel chain.
"""

import os
import numpy as np
from contextlib import ExitStack

BS, J, L, D = 32, 4, 512, 768
K = 4
OUT = 256
NCORES = 8
SPC = BS // NCORES      # samples per core
NE = J * K              # entities per sample
NCH = L // 128          # 128-row L-chunks per (s, j)
CPS = J * NCH           # L-chunks per sample
DC = D // 128           # 128-row D-chunks
DH = D // 2             # free-dim half for pooling matmul (384 <= 512 psum f32)

# matmul operand dtype: "f16" (default: full-rate, half traffic, ~3.5e-4 err),
# "f32r" (full-rate fp32 streaming, ~1.8e-4 err, 2x traffic), "f32", "bf16"
MM_MODE = os.environ.get("BASSK_MM_MODE", "f16")
# alternate the two HWDGE rings (sync/scalar sequencers) for z DMAs
ZRING = int(os.environ.get("BASSK_ZRING", "2"))

_CACHE = {}


def _mm_dt(mybir):
    return {
        "f32r": mybir.dt.float32r,
        "f32": mybir.dt.float32,
        "bf16": mybir.dt.bfloat16,
        "f16": mybir.dt.float16,
    }[MM_MODE]


def _mm_np():
    if MM_MODE == "bf16":
        import ml_dtypes

        return np.dtype(ml_dtypes.bfloat16)
    if MM_MODE == "f16":
        return np.dtype(np.float16)
    return np.dtype(np.float32)


def _build_nc(niter=1, hw_loop=0, nch_sj=None):
    import contextlib

    import concourse.bass as bass
    import concourse.mybir as mybir
    from concourse.bacc import Bacc
    from concourse.tile import TileContext
    from concourse.masks import make_identity

    f32 = mybir.dt.float32
    mmdt = _mm_dt(mybir)
    if nch_sj is None:
        nch_sj = tuple(tuple(NCH for _ in range(J)) for _ in range(SPC))

    nc = Bacc(trn_type="TRN2")
    z = nc.declare_dram_parameter("z", [SPC, J, L, D], mmdt, isOutput=False)
    masks = nc.declare_dram_parameter("masks", [128, SPC * CPS * NE], mmdt, isOutput=False)
    rcount = nc.declare_dram_parameter("rcount", [128, SPC * NE], f32, isOutput=False)
    brep = nc.declare_dram_parameter("brep", [NE, OUT], f32, isOutput=False)
    w = nc.declare_dram_parameter("w", [D, OUT], mmdt, isOutput=False)
    out = nc.declare_dram_parameter("out", [SPC, NE, OUT], f32, isOutput=True)

    with TileContext(nc) as tc:
        with ExitStack() as ctx:
            singles = ctx.enter_context(tc.tile_pool(name="singles", bufs=1))
            zpool = ctx.enter_context(tc.tile_pool(name="zp", bufs=6))
            sums_pool = ctx.enter_context(tc.tile_pool(name="sums", bufs=3))
            ptpool = ctx.enter_context(tc.tile_pool(name="pt", bufs=2))
            outpool = ctx.enter_context(tc.tile_pool(name="outp", bufs=2))
            psum_pool = ctx.enter_context(tc.tile_pool(name="ps", bufs=4, space="PSUM"))
            psum_tp = ctx.enter_context(tc.tile_pool(name="pst", bufs=3, space="PSUM"))
            psum_op = ctx.enter_context(tc.tile_pool(name="pso", bufs=1, space="PSUM"))

            mask_t = singles.tile([128, SPC * CPS * NE], mmdt)
            nc.gpsimd.dma_start(out=mask_t, in_=masks[:, :])
            rc_t = singles.tile([128, SPC * NE], f32)
            nc.gpsimd.dma_start(out=rc_t, in_=rcount[:, :])
            b_t = singles.tile([NE, OUT], f32)
            nc.gpsimd.dma_start(out=b_t, in_=brep[:, :])
            w_t = singles.tile([128, DC, OUT], mmdt)
            nc.gpsimd.dma_start(out=w_t, in_=w.rearrange("(c p) o -> p c o", p=128))
            ident = singles.tile([NE, NE], f32)
            make_identity(nc, ident)

            loop_cm = tc.For_i(0, hw_loop, 1) if hw_loop else contextlib.nullcontext()
            with loop_cm:
              for it in range(niter):
                for s in range(SPC):
                  nchs = nch_sj[s]
                  tot_c = sum(nchs)
                  ps = [psum_pool.tile([NE, DH], f32, name=f"ps{it}_{s}_{h}", tag="ps") for h in range(2)]
                  zviews = []
                  last_it = it == niter - 1 and s == SPC - 1
                  for j in range(J):
                      zt = zpool.tile([128, nchs[j], D], mmdt, name=f"zt{it}_{s}_{j}", tag="zt")
                      if last_it and j == J - 1:
                          for lc in range(nchs[j]):
                              eng = (nc.sync, nc.scalar)[lc % 2] if ZRING > 1 else nc.sync
                              eng.dma_start(
                                  out=zt[:, lc, :],
                                  in_=z[s, j, lc * 128 : (lc + 1) * 128, :],
                              )
                      else:
                          eng = (nc.sync, nc.scalar)[j % 2] if ZRING > 1 else nc.sync
                          eng.dma_start(
                              out=zt,
                              in_=z[s, j, 0 : nchs[j] * 128, :].rearrange(
                                  "(c p) d -> p c d", p=128
                              ),
                          )
                      zviews.append(zt)
                  ci = 0
                  for j in range(J):
                      for lc in range(nchs[j]):
                          c = j * NCH + lc
                          moff = (s * CPS + c) * NE
                          for h in range(2):
                              nc.tensor.matmul(
                                  ps[h][:, :],
                                  lhsT=mask_t[:, moff : moff + NE],
                                  rhs=zviews[j][:, lc, h * DH : (h + 1) * DH],
                                  start=(ci == 0),
                                  stop=(ci == tot_c - 1),
                              )
                          ci += 1
                  sums = sums_pool.tile([NE, D], f32, name=f"sums{it}_{s}", tag="sums")
                  nc.vector.tensor_copy(sums[:, 0:DH], ps[0][:, :])
                  nc.scalar.copy(sums[:, DH : 2 * DH], ps[1][:, :])
                  pt = ptpool.tile([128, DC, NE], mmdt, name=f"pt{it}_{s}", tag="pt")
                  hdc = DC // 2
                  rc_s = rc_t[:, s * NE : (s + 1) * NE]
                  rc_b = bass.AP(
                      tensor=rc_s.tensor,
                      offset=rc_s.offset,
                      ap=[rc_s.ap[0], [0, hdc], rc_s.ap[1]],
                  )
                  for g in range(2):
                      tps = psum_tp.tile(
                          [128, hdc, NE], f32, name=f"tps{it}_{s}_{g}", tag="tp"
                      )
                      for i in range(hdc):
                          dc = g * hdc + i
                          nc.tensor.transpose(
                              tps[:, i, :], sums[:, dc * 128 : (dc + 1) * 128], ident[:, :]
                          )
                      nc.vector.tensor_mul(
                          pt[:, g * hdc : (g + 1) * hdc, :], tps[:, :, :], rc_b
                      )
                  po = psum_op.tile([NE, OUT], f32, name=f"po{it}_{s}", tag="po")
                  for dc in range(DC):
                      nc.tensor.matmul(
                          po[:, :],
                          lhsT=pt[:, dc, :],
                          rhs=w_t[:, dc, :],
                          start=(dc == 0),
                          stop=(dc == DC - 1),
                      )
                  ot = outpool.tile([NE, OUT], f32, name=f"ot{it}_{s}", tag="ot")
                  nc.vector.tensor_add(ot[:, :], po[:, :], b_t[:, :])
                  nc.sync.dma_start(out=out[s, :, :], in_=ot[:, :])
    nc.finalize()
    return nc


def _plan(sep):
    """Per-(sample-slot, sentence) chunk counts + balanced sample->core map."""
    sep_max = np.asarray(sep).max(axis=-1)                       # [BS, J]
    ch = np.ceil(np.maximum(sep_max, 1) / 128.0).astype(int).clip(1, NCH)
    order = np.argsort(-ch.sum(axis=1), kind="stable")
    assign = order.reshape(SPC, NCORES)                          # [slot, core] -> b
    nch_sj = tuple(
        tuple(int(ch[assign[s], j].max()) for j in range(J)) for s in range(SPC)
    )
    return assign, nch_sj


def _prep_in_maps(z, sep, Wf, bf, assign=None):
    # Reference span arithmetic (identical formulas, so edge cases match:
    # count==0 -> 1/0=inf, 0*inf=NaN like the reference's 0/0).
    starts = np.concatenate([np.ones_like(sep[..., :1]), sep[..., :-1] + 1], axis=-1)
    ends = sep
    counts = (ends - starts).astype(np.float32)
    with np.errstate(divide="ignore"):
        rcounts = np.float32(1.0) / counts                      # [BS, J, K]
    t = np.arange(L)
    mask = (
        (t[None, None, None, :] >= starts[..., None])
        & (t[None, None, None, :] < ends[..., None])
    ).astype(np.float32)                                        # [BS, J, K, L]

    mdt = _mm_np()
    brep = np.ascontiguousarray(np.broadcast_to(bf, (NE, OUT)))
    z = z.astype(mdt, copy=False)
    Wm = Wf.astype(mdt, copy=False)
    if assign is None:
        assign = np.arange(BS).reshape(NCORES, SPC).T            # [slot, core]
    in_maps = []
    for c in range(NCORES):
        bsel = assign[:, c]
        # masks layout [p, s, c_in_sample, entity]: chunk c = j*NCH + lc holds
        # tokens l = lc*128 + p of sentence j; entity column e = j*K + k is
        # nonzero only for chunks of its own sentence (block-diagonal in j).
        mc = mask[bsel].reshape(SPC, J, K, NCH, 128)             # [s, j, k, lc, p]
        m2 = mc.transpose(4, 0, 1, 3, 2)                        # [p, s, j, lc, k]
        mfull = np.zeros((128, SPC, CPS, NE), mdt)
        for j in range(J):
            mfull[:, :, j * NCH : (j + 1) * NCH, j * K : (j + 1) * K] = m2[:, :, j]
        in_maps.append(
            {
                "z": np.ascontiguousarray(z[bsel]),
                "masks": np.ascontiguousarray(mfull.reshape(128, SPC * CPS * NE)),
                "rcount": np.ascontiguousarray(
                    np.broadcast_to(
                        rcounts[bsel].reshape(1, SPC * NE), (128, SPC * NE)
                    )
                ),
                "brep": brep,
                "w": Wm,
            }
        )
    return in_maps


def _run(in_maps, nch_sj=None, **kwargs):
    from concourse.bass_utils import run_bass_kernel_spmd

    key = ("nc", nch_sj)
    if key not in _CACHE:
        _CACHE[key] = _build_nc(nch_sj=nch_sj)
    return run_bass_kernel_spmd(_CACHE[key], in_maps, list(range(NCORES)), **kwargs)


def kernel(zipped_entity, entity_token_sep_idx, W, b):
    z = np.ascontiguousarray(np.asarray(zipped_entity, dtype=np.float32))
    sep = np.asarray(entity_token_sep_idx).astype(np.int64)
    Wf = np.ascontiguousarray(np.asarray(W, dtype=np.float32))
    bf = np.asarray(b, dtype=np.float32)
    assert z.shape == (BS, J, L, D) and sep.shape == (BS, J, K)

    assign, nch_sj = _plan(sep)
    res = _run(_prep_in_maps(z, sep, Wf, bf, assign=assign), nch_sj=nch_sj)
    out = np.empty((BS, NE, OUT), np.float32)
    for c in range(NCORES):
        out[assign[:, c]] = res.results[c]["out"]
    return out



# revision 3
# speedup vs baseline: 1.3211x; 1.3211x over previous
"""EntityEncoder Trainium2 kernel.

Computes, for each (batch, sentence j): ragged per-entity span mean-pool over
token embeddings, then a Linear projection:

    pooled[b, j, k, :] = mean(zipped_entity[b, j, start_kj:end_kj, :])
    out[b, j*K+k, :]   = pooled @ W + b

Strategy (8 NeuronCores, data-parallel over batch, memory-bound):
  - Only tokens inside entity spans are ever read.  The host gathers exactly
    those rows (cast to f16) into a dense per-core buffer laid out
    [128, NCHP, D] (partition-major), so the device streams them with a few
    ~1 MB contiguous DMAs at near-peak HBM efficiency.  Samples are
    LPT-balanced across the 8 cores by total span rows; NCHP (the packed
    128-row chunk count) is the only compile-time specialization key.
  - A 0/1 span mask (also host-packed, one 64-entity column block per chunk)
    turns the ragged segment-sum into PE matmuls with z as the *stationary*
    operand:  sumsT[d, e] += z_chunk[:, dblk]^T @ mask_chunk
    accumulated over chunks into 6 PSUM tiles [128, 64] (one per 128-wide
    D block).  Sums land already transposed, so no PE transposes are needed.
  - PSUM eviction multiplies by 1/count (f32, so 1/0=inf reproduces the
    reference's 0/0 -> NaN exactly) while casting to f16:  pt = sumsT * rc.
  - Final Linear contracts D on partitions: out = pt.T @ W (6 accumulating
    matmuls) + bias via one DVE add, then DMA out.
"""

import math
import os
import numpy as np
from contextlib import ExitStack

BS, J, L, D = 32, 4, 512, 768
K = 4
OUT = 256
NCORES = 8
SPC = BS // NCORES      # samples per core
NE = J * K              # entities per sample
CNE = SPC * NE          # entities per core (64)
DC = D // 128           # 128-row D blocks (6)

# chunks per z DMA piece (5 * 128 * 768 * 2B = 983 KB, near the DMA sweet spot)
PIECE = int(os.environ.get("BASSK_PIECE", "5"))
MASK_DT = os.environ.get("BASSK_MASK_DT", "f16")  # f16 | fp8

_CACHE = {}


def _mask_dt(mybir):
    return {"f16": mybir.dt.float16, "fp8": mybir.dt.float8e4}[MASK_DT]


def _mask_np():
    if MASK_DT == "fp8":
        import ml_dtypes

        return np.dtype(ml_dtypes.float8_e4m3)
    return np.dtype(np.float16)


def _build_nc(niter=1, hw_loop=0, nch_sj=None):
    import contextlib

    import concourse.bass as bass  # noqa: F401
    import concourse.mybir as mybir
    from concourse.bacc import Bacc
    from concourse.tile import TileContext

    f32 = mybir.dt.float32
    f16 = mybir.dt.float16
    mkdt = _mask_dt(mybir)
    nchp = int(nch_sj) if nch_sj is not None else 17

    nc = Bacc(trn_type="TRN2")
    z = nc.declare_dram_parameter("z", [128, nchp, D], f16, isOutput=False)
    masks = nc.declare_dram_parameter("masks", [128, nchp, CNE], mkdt, isOutput=False)
    rcount = nc.declare_dram_parameter("rcount", [128, CNE], f32, isOutput=False)
    brep = nc.declare_dram_parameter("brep", [CNE, OUT], f32, isOutput=False)
    w = nc.declare_dram_parameter("w", [128, DC, OUT], f16, isOutput=False)
    out = nc.declare_dram_parameter("out", [CNE, OUT], f32, isOutput=True)

    pieces = []
    c0 = 0
    while c0 < nchp:
        c1 = min(c0 + PIECE, nchp)
        pieces.append((c0, c1))
        c0 = c1

    with TileContext(nc) as tc:
        with ExitStack() as ctx:
            zpool = ctx.enter_context(tc.tile_pool(name="zp", bufs=2))
            mpool = ctx.enter_context(tc.tile_pool(name="mp", bufs=2))
            spool = ctx.enter_context(tc.tile_pool(name="sp", bufs=2))
            ptpool = ctx.enter_context(tc.tile_pool(name="pt", bufs=2))
            outpool = ctx.enter_context(tc.tile_pool(name="outp", bufs=2))
            psum_pool = ctx.enter_context(tc.tile_pool(name="ps", bufs=1, space="PSUM"))

            loop_cm = tc.For_i(0, hw_loop, 1) if hw_loop else contextlib.nullcontext()
            with loop_cm:
              for it in range(niter):
                mask_t = mpool.tile([128, nchp, CNE], mkdt, name=f"mk{it}", tag="mk")
                nc.scalar.dma_start(out=mask_t, in_=masks[:, :, :])
                zt = zpool.tile([128, nchp, D], f16, name=f"zt{it}", tag="zt")
                for pi, (c0, c1) in enumerate(pieces):
                    eng = (nc.sync, nc.scalar)[pi % 2]
                    eng.dma_start(out=zt[:, c0:c1, :], in_=z[:, c0:c1, :])
                w_t = spool.tile([128, DC, OUT], f16, name=f"w{it}", tag="w")
                nc.gpsimd.dma_start(out=w_t, in_=w[:, :, :])
                rc_t = spool.tile([128, CNE], f32, name=f"rc{it}", tag="rc")
                nc.gpsimd.dma_start(out=rc_t, in_=rcount[:, :])
                b_t = spool.tile([CNE, OUT], f32, name=f"b{it}", tag="b")
                nc.gpsimd.dma_start(out=b_t, in_=brep[:, :])

                ps = [
                    psum_pool.tile([128, CNE], f32, name=f"ps{it}_{d}", tag=f"ps{d}")
                    for d in range(DC)
                ]
                for c in range(nchp):
                    for d in range(DC):
                        nc.tensor.matmul(
                            ps[d][:, :],
                            lhsT=zt[:, c, d * 128 : (d + 1) * 128],
                            rhs=mask_t[:, c, :],
                            start=(c == 0),
                            stop=(c == nchp - 1),
                        )
                pt = ptpool.tile([128, DC, CNE], f16, name=f"pt{it}", tag="pt")
                po = psum_pool.tile([CNE, OUT], f32, name=f"po{it}", tag="po")
                for d in range(DC):
                    nc.vector.tensor_mul(pt[:, d, :], ps[d][:, :], rc_t[:, :])
                    nc.tensor.matmul(
                        po[:, :],
                        lhsT=pt[:, d, :],
                        rhs=w_t[:, d, :],
                        start=(d == 0),
                        stop=(d == DC - 1),
                    )
                ot = outpool.tile([CNE, OUT], f32, name=f"ot{it}", tag="ot")
                nc.vector.tensor_add(ot[:, :], po[:, :], b_t[:, :])
                nc.sync.dma_start(out=out[:, :], in_=ot[:, :])
    nc.finalize()
    return nc


def _spans(sep):
    sep = np.asarray(sep, dtype=np.int64)
    starts = np.concatenate([np.ones_like(sep[..., :1]), sep[..., :-1] + 1], axis=-1)
    counts = sep - starts                                        # [BS, J, K]
    return starts, counts


def _plan(sep):
    """LPT-balance samples over cores by total span rows; nchp compile key."""
    _, counts = _spans(sep)
    rows_b = np.clip(counts, 0, None).sum(axis=(1, 2))           # [BS]
    order = np.argsort(-rows_b, kind="stable")
    loads = np.zeros(NCORES, dtype=np.int64)
    slots = [[] for _ in range(NCORES)]
    for b in order:
        cands = [i for i in range(NCORES) if len(slots[i]) < SPC]
        c = min(cands, key=lambda i: loads[i])
        slots[c].append(int(b))
        loads[c] += int(rows_b[b])
    assign = np.array(slots, dtype=np.int64).T                   # [slot, core] -> b
    nchp = max(1, int(math.ceil(loads.max() / 128.0)))
    return assign, nchp


def _prep_in_maps(z, sep, Wf, bf, assign=None, nchp=None):
    starts, counts = _spans(sep)
    if assign is None:
        assign = np.arange(BS).reshape(NCORES, SPC).T
    if nchp is None:
        rows_b = np.clip(counts, 0, None).sum(axis=(1, 2))
        nchp = max(
            1, int(math.ceil(max(rows_b[assign[:, c]].sum() for c in range(NCORES)) / 128.0))
        )

    mdt = np.float16
    mkdt = _mask_np()
    w128 = np.ascontiguousarray(
        Wf.reshape(DC, 128, OUT).transpose(1, 0, 2).astype(mdt)
    )
    brep = np.ascontiguousarray(np.broadcast_to(bf, (CNE, OUT)).astype(np.float32))

    with np.errstate(divide="ignore"):
        rcounts = np.float32(1.0) / counts.astype(np.float32)    # [BS, J, K]

    in_maps = []
    for c in range(NCORES):
        bsel = assign[:, c]
        segs = []          # (packed_start, n, entity)
        r = 0
        for s, b in enumerate(bsel):
            for j in range(J):
                for k in range(K):
                    n = int(counts[b, j, k])
                    if n > 0:
                        segs.append((r, n, s * NE + j * K + k, b, j, k))
                        r += n
        rows = np.zeros((nchp * 128, D), mdt)
        mrows = np.zeros((nchp * 128, CNE), mkdt)
        for r0, n, e, b, j, k in segs:
            s0 = int(starts[b, j, k])
            rows[r0 : r0 + n] = z[b, j, s0 : s0 + n, :]
            mrows[r0 : r0 + n, e] = 1.0
        zp = np.ascontiguousarray(rows.reshape(nchp, 128, D).transpose(1, 0, 2))
        mk = np.ascontiguousarray(mrows.reshape(nchp, 128, CNE).transpose(1, 0, 2))
        rcc = rcounts[bsel].reshape(CNE)                          # [64]
        in_maps.append(
            {
                "z": zp,
                "masks": mk,
                "rcount": np.ascontiguousarray(
                    np.broadcast_to(rcc, (128, CNE)).astype(np.float32)
                ),
                "brep": brep,
                "w": w128,
            }
        )
    return in_maps


def _run(in_maps, nch_sj=None, **kwargs):
    from concourse.bass_utils import run_bass_kernel_spmd

    key = ("nc", nch_sj)
    if key not in _CACHE:
        _CACHE[key] = _build_nc(nch_sj=nch_sj)
    return run_bass_kernel_spmd(_CACHE[key], in_maps, list(range(NCORES)), **kwargs)


def kernel(zipped_entity, entity_token_sep_idx, W, b):
    z = np.asarray(zipped_entity, dtype=np.float32)
    sep = np.asarray(entity_token_sep_idx).astype(np.int64)
    Wf = np.ascontiguousarray(np.asarray(W, dtype=np.float32))
    bf = np.asarray(b, dtype=np.float32)
    assert z.shape == (BS, J, L, D) and sep.shape == (BS, J, K)

    assign, nchp = _plan(sep)
    res = _run(_prep_in_maps(z, sep, Wf, bf, assign=assign, nchp=nchp), nch_sj=nchp)
    out = np.empty((BS, NE, OUT), np.float32)
    for c in range(NCORES):
        out[assign[:, c]] = res.results[c]["out"].reshape(SPC, NE, OUT)
    return out


# revision 16
# speedup vs baseline: 1.4674x; 1.1108x over previous
"""EntityEncoder Trainium2 kernel.

Computes, for each (batch, sentence j): ragged per-entity span mean-pool over
token embeddings, then a Linear projection:

    pooled[b, j, k, :] = mean(zipped_entity[b, j, start_kj:end_kj, :])
    out[b, j*K+k, :]   = pooled @ W + b

Strategy (8 NeuronCores, data-parallel over batch, memory-bound):
  - Only tokens inside entity spans are ever read.  The host gathers exactly
    those rows (cast to f16) into a dense per-core buffer laid out
    [128, NCHP, D] (partition-major), so the device streams them with a few
    ~1 MB contiguous DMAs at near-peak HBM efficiency.  Samples are
    LPT-balanced across the 8 cores by total span rows; NCHP (the packed
    128-row chunk count) is the only compile-time specialization key.
  - A span mask holding 1/count (host-packed, one 64-entity column block per
    chunk, f16) turns the ragged segment-MEAN into PE matmuls with z as the
    *stationary* operand:  pooledT[d, e] += z_chunk[:, dblk]^T @ mask_chunk
    accumulated over chunks into 6 PSUM tiles [128, 64].  Sums land already
    transposed and already divided, so eviction is a plain PSUM->SBUF copy
    (split across DVE and ACT) and no PE transposes are needed.
  - masks + z pieces go on the sync HWDGE ring in exact consumption order
    (no head-of-line blocking); W/bias ride the gpsimd SWDGE ring.
  - Final Linear contracts D on partitions: out = ptT.T @ W via 6
    accumulating matmuls plus a 1-row ones@bias matmul, evicted in two
    halves overlapped with the two output DMAs.
  - count==0 entities (possible under duplicate separator indices) are fixed
    up host-side to NaN rows, matching the reference's 0/0 -> NaN.
"""

import math
import os
import numpy as np
from contextlib import ExitStack

BS, J, L, D = 32, 4, 512, 768
K = 4
OUT = 256
NCORES = 8
SPC = BS // NCORES      # samples per core
NE = J * K              # entities per sample
CNE = SPC * NE          # entities per core (64)
DC = D // 128           # 128-row D blocks (6)

# chunks per z DMA piece (5 * 128 * 768 * 2B = 983 KB, near the DMA sweet spot)
PIECE = int(os.environ.get("BASSK_PIECE", "5"))


_CACHE = {}


def _build_nc(niter=1, hw_loop=0, nch_sj=None):
    import contextlib

    import concourse.bass as bass  # noqa: F401
    import concourse.mybir as mybir
    from concourse.bacc import Bacc
    from concourse.tile import TileContext

    f32 = mybir.dt.float32
    f16 = mybir.dt.float16
    mkdt = f16
    nchp = int(nch_sj) if nch_sj is not None else 17

    nc = Bacc(trn_type="TRN2")
    z = nc.declare_dram_parameter("z", [128, nchp, D], f16, isOutput=False)
    masks = nc.declare_dram_parameter("masks", [128, nchp, CNE], mkdt, isOutput=False)
    bvec = nc.declare_dram_parameter("bvec", [1, OUT], f16, isOutput=False)
    w = nc.declare_dram_parameter("w", [128, DC, OUT], f16, isOutput=False)
    out = nc.declare_dram_parameter("out", [CNE, OUT], f32, isOutput=True)

    pieces = []
    c0 = 0
    while c0 < nchp:
        c1 = min(c0 + PIECE, nchp)
        if c1 == nchp and c1 - c0 > 1:
            # keep the final piece tiny so the last accumulation chunk (and
            # everything downstream of it) starts as early as possible
            pieces.append((c0, c1 - 1))
            c0 = c1 - 1
        else:
            pieces.append((c0, c1))
            c0 = c1

    with TileContext(nc) as tc:
        with ExitStack() as ctx:
            singles = ctx.enter_context(tc.tile_pool(name="sing", bufs=1))
            zpool = ctx.enter_context(tc.tile_pool(name="zp", bufs=2))
            mpool = ctx.enter_context(tc.tile_pool(name="mp", bufs=2))
            spool = ctx.enter_context(tc.tile_pool(name="sp", bufs=2))
            ptpool = ctx.enter_context(tc.tile_pool(name="pt", bufs=2))
            outpool = ctx.enter_context(tc.tile_pool(name="outp", bufs=2))
            psum_pool = ctx.enter_context(tc.tile_pool(name="ps", bufs=1, space="PSUM"))

            ones_t = singles.tile([1, CNE], f16)
            nc.vector.memset(ones_t[:, :], 1.0)

            loop_cm = tc.For_i(0, hw_loop, 1) if hw_loop else contextlib.nullcontext()
            with loop_cm:
              for it in range(niter):
                b_t = spool.tile([1, OUT], f16, name=f"b{it}", tag="b")
                nc.scalar.dma_start(out=b_t, in_=bvec[:, :])
                mask_t = mpool.tile([128, nchp, CNE], mkdt, name=f"mk{it}", tag="mk")
                nc.sync.dma_start(out=mask_t, in_=masks[:, :, :])
                zt = zpool.tile([128, nchp, D], f16, name=f"zt{it}", tag="zt")
                for c0, c1 in pieces:
                    nc.sync.dma_start(out=zt[:, c0:c1, :], in_=z[:, c0:c1, :])
                w_t = spool.tile([128, DC, OUT], f16, name=f"w{it}", tag="w")
                nc.gpsimd.dma_start(out=w_t, in_=w[:, :, :])

                ps = [
                    psum_pool.tile([128, CNE], f32, name=f"ps{it}_{d}", tag=f"ps{d}")
                    for d in range(DC)
                ]
                for c in range(nchp):
                    for d in range(DC):
                        nc.tensor.matmul(
                            ps[d][:, :],
                            lhsT=zt[:, c, d * 128 : (d + 1) * 128],
                            rhs=mask_t[:, c, :],
                            start=(c == 0),
                            stop=(c == nchp - 1),
                        )
                pt = ptpool.tile([128, DC, CNE], f16, name=f"pt{it}", tag="pt")
                po = psum_pool.tile([CNE, OUT], f32, name=f"po{it}", tag="po")
                for d in range(DC):
                    if d % 2 == 0:
                        nc.vector.tensor_copy(pt[:, d, :], ps[d][:, :])
                    else:
                        nc.scalar.copy(pt[:, d, :], ps[d][:, :])
                    nc.tensor.matmul(
                        po[:, :],
                        lhsT=pt[:, d, :],
                        rhs=w_t[:, d, :],
                        start=(d == 0),
                        stop=False,
                    )
                nc.tensor.matmul(
                    po[:, :],
                    lhsT=ones_t[:, :],
                    rhs=b_t[:, :],
                    start=False,
                    stop=True,
                )
                ot = outpool.tile([CNE, OUT], f32, name=f"ot{it}", tag="ot")
                nc.vector.tensor_copy(ot[:, 0 : OUT // 2], po[:, 0 : OUT // 2])
                nc.scalar.copy(ot[:, OUT // 2 : OUT], po[:, OUT // 2 : OUT])
                nc.scalar.dma_start(out=out[:, :], in_=ot[:, :])
    nc.finalize()
    return nc


def _spans(sep):
    sep = np.asarray(sep, dtype=np.int64)
    starts = np.concatenate([np.ones_like(sep[..., :1]), sep[..., :-1] + 1], axis=-1)
    counts = sep - starts                                        # [BS, J, K]
    return starts, counts


def _plan(sep):
    """LPT-balance the 512 independent entities over cores (64 each) by span
    rows; nchp (max per-core packed 128-chunks) is the compile key."""
    _, counts = _spans(sep)
    rows_e = np.clip(counts, 0, None).reshape(-1)                # [BS*NE]
    order = np.argsort(-rows_e, kind="stable")
    loads = np.zeros(NCORES, dtype=np.int64)
    slots = [[] for _ in range(NCORES)]
    for e in order:
        cands = [i for i in range(NCORES) if len(slots[i]) < CNE]
        c = min(cands, key=lambda i: loads[i])
        slots[c].append(int(e))
        loads[c] += int(rows_e[e])
    assign = np.array(slots, dtype=np.int64)                     # [core, slot] -> eg
    nchp = max(1, int(math.ceil(loads.max() / 128.0)))
    return assign, nchp


def _prep_in_maps(z, sep, Wf, bf, assign=None, nchp=None):
    starts, counts = _spans(sep)
    if assign is None or nchp is None:
        assign, nchp = _plan(sep)
    starts_e = starts.reshape(-1)
    counts_e = counts.reshape(-1)

    mdt = np.float16
    w128 = np.ascontiguousarray(
        Wf.reshape(DC, 128, OUT).transpose(1, 0, 2).astype(mdt)
    )

    zf = z.reshape(BS * NE // K, L // 1, D) if False else z
    in_maps = []
    for c in range(NCORES):
        rows = np.zeros((nchp * 128, D), mdt)
        mrows = np.zeros((nchp * 128, CNE), mdt)
        r = 0
        for slot, eg in enumerate(assign[c]):
            n = int(counts_e[eg])
            if n <= 0:
                continue
            b, jk = divmod(int(eg), NE)
            j = jk // K
            s0 = int(starts_e[eg])
            rows[r : r + n] = z[b, j, s0 : s0 + n, :]
            mrows[r : r + n, slot] = np.float16(1.0 / n)
            r += n
        zp = np.ascontiguousarray(rows.reshape(nchp, 128, D).transpose(1, 0, 2))
        mk = np.ascontiguousarray(mrows.reshape(nchp, 128, CNE).transpose(1, 0, 2))
        in_maps.append({"z": zp, "masks": mk, "bvec": bvec, "w": w128})
    return in_maps


def _run(in_maps, nch_sj=None, **kwargs):
    from concourse.bass_utils import run_bass_kernel_spmd

    key = ("nc", nch_sj)
    if key not in _CACHE:
        _CACHE[key] = _build_nc(nch_sj=nch_sj)
    return run_bass_kernel_spmd(_CACHE[key], in_maps, list(range(NCORES)), **kwargs)


def kernel(zipped_entity, entity_token_sep_idx, W, b):
    z = np.asarray(zipped_entity, dtype=np.float32)
    sep = np.asarray(entity_token_sep_idx).astype(np.int64)
    Wf = np.ascontiguousarray(np.asarray(W, dtype=np.float32))
    bf = np.asarray(b, dtype=np.float32)
    assert z.shape == (BS, J, L, D) and sep.shape == (BS, J, K)

    assign, nchp = _plan(sep)
    res = _run(_prep_in_maps(z, sep, Wf, bf, assign=assign, nchp=nchp), nch_sj=nchp)
    out = np.empty((BS * NE, OUT), np.float32)
    for c in range(NCORES):
        out[assign[c]] = res.results[c]["out"]
    out = out.reshape(BS, NE, OUT)
    _, counts = _spans(sep)
    zb, zj, zk = np.nonzero(counts == 0)
    for b_, j_, k_ in zip(zb, zj, zk):
        out[b_, j_ * K + k_, :] = np.nan
    return out
